# revision 17
# baseline (speedup 1.0000x reference)
"""AAM (additive angular margin) loss on 8 TRN2 NeuronCores.

loss = mean_r [ logsumexp_c(30 * (x_hat[r,c] - 0.5*onehot(label_r))) - 30*(x_hat[r,label_r] - 0.5) ]
with x_hat = x / max(||x||_2, 1e-12) per row.

Strategy: shard rows across 8 cores (1024 rows each). The host casts each
core's [1024, 32000] shard to bf16 before upload, halving HBM traffic (the
memory roofline) from 131MB to 65.5MB per core; the 2e-2 harness tolerance
dwarfs the ~1e-4 bf16 quantization effect on the loss. Each core streams
its bf16 shard from HBM exactly once (8 row-blocks of 128 partitions, in
column chunks resident in SBUF between the two passes).

Engine budget per 128-row block (32000 elems/partition-row):
  ACT (the bottleneck): exp(sca*x) with accum_out over every element is
    1 elem/cycle/lane dtype-independent (~26.7us) + a ~2.4K-elem Square
    slice of pass 1 to offload the slower VectorE (+2us).
  DVE: pass-1 sum(x^2) via scalar_tensor_tensor with accum_out runs at 1x
    (no 2x/4x DVE uop exists for ANY accumulating op - HW-verified: plain
    TT hits 2x on bf16, TENSOR_SCALAR_CACHE_REDUCE stays 1x), ~29.6K
    elems ~= 31us. Both engines land at ~31us/block.
  The whole per-row scale chain stays on ACT to avoid cross-engine
    ping-pong: Identity-with-accum reduces the per-chunk partials,
    ln(ss + 1e-24) (the F.normalize eps clamp folded into the ACT bias
    port), then sca = 30/sqrt(ss) = exp(-0.5*ln + ln30).
  ACT activation tables: bass pins each activation function to the first
    act_info.json set containing it, which makes Ln<->Exp alternation
    reload tables ~2x per block (~2.7us each). natural_log_exp_and_others
    holds ALL functions used here (exp/ln/square/identity), so build()
    patches the table registry handed to bacc's load-insertion pass to
    pin everything to that one set: exactly one ACT_TABLE_LOAD per run.
  Block 0 uses finer chunks (DVE) plus a bigger ACT share so pass 1
    trails the very first DMAs; the first big exp starts ~31us in.
  All label/margin correction math is batched AFTER the block loop (one
    FD=8 pass per op) - per-block corrections would stall ACT behind
    3.5us-a-pop GpSimd scalar ops.
The margin term needs only x[r, label_r], gathered once per core with a
1024-element indirect DMA; the label column of the softmax sum is corrected
analytically: S' = S - exp(30t) + exp(30t - 15), t = x_label/||x||.
nll = ln(S') - (30t - 15).  Per-core scalar partial via a [128,1]x[128,1]
matmul against a 1/N vector; the host unshard sums the 8 per-core partials
(a device-side AllReduce of the 4-byte scalar costs ~55us of ncfw floor).
"""

import math

import numpy as np

MARGIN = 0.5
SCALE = 30.0
N_CORES = 8
N_TOTAL = 8192
C = 32000
P = 128

R = N_TOTAL // N_CORES  # rows per core
B = R // P  # row blocks per core

# per-block column plan: list of (engine, offset, width); 'a' = ACT Square,
# 'v' = VectorE stt.  DMA issue order == list order.  The ACT chunk goes
# LAST: it lands at the end of the block's DMA window, exactly when ACT
# (running one block behind) gets to it - so the ss chain never waits on
# a late DVE stt of a late-landing chunk.
_STEADY = [
    ("v", 0, 10200),
    ("v", 10200, 10200),
    ("v", 20400, 10000),
    ("a", 30400, 1600),
]
# block 0: v-chunks first (DVE starts as soon as each lands), ACT sweeps the
# back half while the initial DMA stream finishes
_BLOCK0 = [
    ("v", 0, 8000),
    ("v", 8000, 6400),
    ("v", 14400, 1600),
    ("a", 16000, 3200),
    ("a", 19200, 3200),
    ("a", 22400, 3200),
    ("a", 25600, 3200),
    ("a", 28800, 3200),
]
SPANS_BY_BLOCK = [_BLOCK0] + [_STEADY] * (B - 1)


def _pin_act_tables(bacc_mod, mybir):
    """Patch the activation-table registry handed to bacc's table-load
    insertion so every function this kernel uses pins to the one set that
    contains them all (natural_log_exp_and_others). Set ids (dict order)
    are preserved; other sets merely stop advertising these functions."""
    AF = mybir.ActivationFunctionType
    orig = bacc_mod.get_activation_tables
    if getattr(orig, "_aam_pinned", False):
        return
    pinned_funcs = {AF.Exp, AF.Ln, AF.Square, AF.Identity}
    keep = "natural_log_exp_and_others"

    def patched(arch):
        t = dict(orig(arch))
        if keep in t:
            for k in t:
                if k != keep:
                    t[k] = set(t[k]) - pinned_funcs
        return t

    patched._aam_pinned = True
    bacc_mod.get_activation_tables = patched


def build(
    n_rows=R,
    n_cols=C,
    n_cores=N_CORES,
    n_total=N_TOTAL,
    v_bufs=6,
    a_bufs=6,
):
    """Build + compile the per-core Bass graph (SPMD, identical on all cores)."""
    import concourse.bacc as bacc
    import concourse.bass as bass
    import concourse.tile as tile
    from concourse import mybir

    f32 = mybir.dt.float32
    bf16 = mybir.dt.bfloat16
    u32 = mybir.dt.uint32
    AF = mybir.ActivationFunctionType
    ALU = mybir.AluOpType
    AX = mybir.AxisListType

    _pin_act_tables(bacc, mybir)

    b_blocks = n_rows // P
    assert n_rows % P == 0
    for spans in SPANS_BY_BLOCK:
        assert sum(w for _, _, w in spans) == n_cols
    n_es = sum(len(s) for s in SPANS_BY_BLOCK)
    v_max = max(w for s in SPANS_BY_BLOCK for e, _, w in s if e == "v")
    a_max = max(w for s in SPANS_BY_BLOCK for e, _, w in s if e == "a")

    nc = bacc.Bacc("TRN2", target_bir_lowering=False, debug=False, num_devices=n_cores)

    logits_ext = nc.dram_tensor("logits", [n_rows, n_cols], bf16, kind="ExternalInput")
    goff_ext = nc.dram_tensor("goff", [P, b_blocks], u32, kind="ExternalInput")
    out_ext = nc.dram_tensor("out", [1, 1], f32, kind="ExternalOutput")

    neg_m = -SCALE * MARGIN  # -15
    ln_s = math.log(SCALE)

    with tile.TileContext(nc) as tc:
        with (
            tc.tile_pool(name="chunks", bufs=1) as chunks,
            tc.tile_pool(name="singles", bufs=1) as singles,
            tc.tile_pool(name="smalls", bufs=3) as smalls,
            tc.tile_pool(name="ppool", bufs=1, space="PSUM") as ppool,
        ):
            # label-logit gather: one indirect DMA for all rows of this core
            # (goff via gpsimd/SWDGE so the sync HWDGE queue leads with the
            # first streaming chunk)
            goff_sb = singles.tile([P, b_blocks], u32)
            nc.gpsimd.dma_start(out=goff_sb[:, :], in_=goff_ext[:, :])
            xl_all = singles.tile([P, b_blocks], bf16)
            logits_flat = logits_ext.ap().rearrange("r (c one) -> (r c) one", one=1)
            nc.gpsimd.indirect_dma_start(
                out=xl_all[:, :],
                out_offset=None,
                in_=logits_flat,
                in_offset=bass.IndirectOffsetOnAxis(ap=goff_sb[:, :], axis=0),
            )

            zero_t = singles.tile([P, 1], f32)
            nc.vector.memset(zero_t, 0.0)
            m15_t = singles.tile([P, 1], f32)
            nc.vector.memset(m15_t, neg_m)
            ln30_t = singles.tile([P, 1], f32)
            nc.vector.memset(ln30_t, ln_s)
            eps2_t = singles.tile([P, 1], f32)
            nc.vector.memset(eps2_t, 1e-24)
            invn_t = singles.tile([P, 1], f32)
            nc.vector.memset(invn_t, 1.0 / n_total)

            # persistent per-block state for the batched tail
            sca_all = singles.tile([P, b_blocks], f32)
            es_all = singles.tile([P, n_es], f32)
            # stt needs a full-size dummy out (never read)
            dump_v = singles.tile([P, v_max], bf16)
            dump_a = singles.tile([P, a_max], bf16)

            es_base = 0
            for b, spans in enumerate(SPANS_BY_BLOCK):
                ncol = len(spans)
                rs = b * P
                ss_cols = smalls.tile([P, ncol], f32, tag="ss_cols", name=f"ssc_{b}")
                chs = []
                for eng, off, w in spans:
                    # rings by width class so SBUF isn't wasted on padding
                    if w > 3200:
                        tag, bufs, wmax = "vch", v_bufs, 10200
                    else:
                        tag, bufs, wmax = "ach", a_bufs, 3200
                    ch = chunks.tile([P, wmax], bf16, tag=tag, bufs=bufs, name=f"c{b}_{off}")
                    nc.sync.dma_start(
                        out=ch[:, :w], in_=logits_ext[rs : rs + P, off : off + w]
                    )
                    chs.append((eng, ch, w))
                # pass 1: ss_cols[:, i] = sum(chunk^2)
                for i, (eng, ch, w) in enumerate(chs):
                    if eng == "a":
                        nc.scalar.activation(
                            out=dump_a[:, :w],
                            in_=ch[:, :w],
                            func=AF.Square,
                            bias=zero_t[:, :],
                            accum_out=ss_cols[:, i : i + 1],
                        )
                    else:
                        nc.vector.scalar_tensor_tensor(
                            out=dump_v[:, :w],
                            in0=ch[:, :w],
                            scalar=1.0,
                            in1=ch[:, :w],
                            op0=ALU.mult,
                            op1=ALU.mult,
                            accum_out=ss_cols[:, i : i + 1],
                        )

                # whole scale chain on ACT (no cross-engine ping-pong):
                # ss = sum(ss_cols); u = ln(ss + eps^2); sca = exp(-u/2 + ln30)
                ss_dump = smalls.tile([P, ncol], f32, tag="ss_dump", name=f"ssd_{b}")
                ss = smalls.tile([P, 1], f32, tag="ss")
                nc.scalar.activation(
                    out=ss_dump[:, :],
                    in_=ss_cols[:, :],
                    func=AF.Identity,
                    bias=zero_t[:, :],
                    accum_out=ss[:, :],
                )
                u = smalls.tile([P, 1], f32, tag="u")
                nc.scalar.activation(out=u[:, :], in_=ss[:, :], func=AF.Ln, bias=eps2_t[:, :])
                nc.scalar.activation(
                    out=sca_all[:, b : b + 1],
                    in_=u[:, :],
                    func=AF.Exp,
                    bias=ln30_t[:, :],
                    scale=-0.5,
                )

                # pass 2: es_all[:, es_base+i] = sum(exp(sca * x)), in place
                for i, (eng, ch, w) in enumerate(chs):
                    col = es_base + i
                    nc.scalar.activation(
                        out=ch[:, :w],
                        in_=ch[:, :w],
                        func=AF.Exp,
                        bias=zero_t[:, :],
                        scale=sca_all[:, b : b + 1],
                        accum_out=es_all[:, col : col + 1],
                    )
                es_base += ncol

            # ---- batched tail: margin/label correction for all blocks ----
            s_sum = singles.tile([P, b_blocks], f32)
            es_base = 0
            for b, spans in enumerate(SPANS_BY_BLOCK):
                nc.vector.reduce_sum(
                    out=s_sum[:, b : b + 1],
                    in_=es_all[:, es_base : es_base + len(spans)],
                    axis=AX.X,
                )
                es_base += len(spans)
            # t30 = 30 * x_label / ||x||
            t30 = singles.tile([P, b_blocks], f32)
            nc.vector.tensor_tensor(
                out=t30[:, :], in0=xl_all[:, :], in1=sca_all[:, :], op=ALU.mult
            )
            e1 = singles.tile([P, b_blocks], f32)
            nc.scalar.activation(out=e1[:, :], in_=t30[:, :], func=AF.Exp, bias=zero_t[:, :])
            e2 = singles.tile([P, b_blocks], f32)
            nc.scalar.activation(out=e2[:, :], in_=t30[:, :], func=AF.Exp, bias=m15_t[:, :])
            # sc = s_sum - e1 + e2  (replace label term with margined one)
            sc1 = singles.tile([P, b_blocks], f32)
            nc.vector.scalar_tensor_tensor(
                out=sc1[:, :], in0=e1[:, :], scalar=-1.0, in1=s_sum[:, :],
                op0=ALU.mult, op1=ALU.add,
            )
            sc2 = singles.tile([P, b_blocks], f32)
            nc.vector.tensor_tensor(out=sc2[:, :], in0=sc1[:, :], in1=e2[:, :], op=ALU.add)
            lse = singles.tile([P, b_blocks], f32)
            nc.scalar.activation(out=lse[:, :], in_=sc2[:, :], func=AF.Ln, bias=zero_t[:, :])
            # nll = lse - t30 + 15
            nll0 = singles.tile([P, b_blocks], f32)
            nc.vector.scalar_tensor_tensor(
                out=nll0[:, :], in0=t30[:, :], scalar=-1.0, in1=lse[:, :],
                op0=ALU.mult, op1=ALU.add,
            )
            nll1 = singles.tile([P, b_blocks], f32)
            nc.vector.tensor_scalar(
                out=nll1[:, :], in0=nll0[:, :], scalar1=-neg_m, scalar2=None, op0=ALU.add
            )

            # per-core scalar: sum_p sum_b nll / n_total  (partition reduce by matmul)
            nll_row = singles.tile([P, 1], f32)
            nc.vector.reduce_sum(out=nll_row[:, :], in_=nll1[:, :], axis=AX.X)
            pt = ppool.tile([1, 1], f32)
            nc.tensor.matmul(
                out=pt[:, :], lhsT=nll_row[:, :], rhs=invn_t[:, :], start=True, stop=True
            )
            final_sb = singles.tile([1, 1], f32)
            nc.vector.tensor_copy(out=final_sb[:, :], in_=pt[:, :])
            nc.sync.dma_start(out=out_ext[:, :], in_=final_sb[:, :])

    nc.compile()
    return nc


_NC_CACHE = None


def _get_nc():
    global _NC_CACHE
    if _NC_CACHE is None:
        _NC_CACHE = build()
    return _NC_CACHE


def make_in_maps(logits, labels):
    import ml_dtypes

    logits = np.asarray(logits, dtype=np.float32)
    labels = np.asarray(labels).astype(np.int64)
    assert logits.shape == (N_TOTAL, C), logits.shape
    logits_bf16 = logits.astype(ml_dtypes.bfloat16)
    in_maps = []
    for i in range(N_CORES):
        shard = np.ascontiguousarray(logits_bf16[i * R : (i + 1) * R])
        lab = labels[i * R : (i + 1) * R]
        flat = np.arange(R, dtype=np.int64) * C + lab  # local flat element index
        goff = np.ascontiguousarray(flat.reshape(B, P).T).astype(np.uint32)
        in_maps.append({"logits": shard, "goff": goff})
    return in_maps


def kernel(**inputs):
    from concourse.bass_utils import run_bass_kernel_spmd

    nc = _get_nc()
    in_maps = make_in_maps(inputs["logits"], inputs["labels"])
    res = run_bass_kernel_spmd(nc, in_maps, core_ids=list(range(N_CORES)))
    # each core emits its shard's nll-sum / N_TOTAL; unshard = sum of partials
    total = sum(float(np.asarray(r["out"]).reshape(())) for r in res.results)
    return np.array(total, dtype=np.float32)


# revision 26
# speedup vs baseline: 1.0291x; 1.0291x over previous
"""AAM (additive angular margin) loss on 8 TRN2 NeuronCores.

loss = mean_r [ logsumexp_c(30 * (x_hat[r,c] - 0.5*onehot(label_r))) - 30*(x_hat[r,label_r] - 0.5) ]
with x_hat = x / max(||x||_2, 1e-12) per row.

Strategy: shard rows across 8 cores (1024 rows each). The host casts each
core's [1024, 32000] shard to bf16 before upload, halving HBM traffic (the
memory roofline) from 131MB to 65.5MB per core; the 2e-2 harness tolerance
dwarfs the ~1e-4 bf16 quantization effect on the loss. Each core streams
its bf16 shard from HBM exactly once (8 row-blocks of 128 partitions, in
column chunks resident in SBUF between the two passes).

Engine budget per 128-row block (32000 elems/partition-row):
  ACT (the bottleneck): exp(sca*x) with accum_out over every element is
    1 elem/cycle/lane dtype-independent (~26.7us) + a ~2.4K-elem Square
    slice of pass 1 to offload the slower VectorE (+2us).
  DVE: pass-1 sum(x^2) via scalar_tensor_tensor with accum_out runs at 1x
    (no 2x/4x DVE uop exists for ANY accumulating op - HW-verified: plain
    TT hits 2x on bf16, TENSOR_SCALAR_CACHE_REDUCE stays 1x), ~29.6K
    elems ~= 31us. Both engines land at ~31us/block.
  The whole per-row scale chain stays on ACT to avoid cross-engine
    ping-pong: Identity-with-accum reduces the per-chunk partials,
    ln(ss + 1e-24) (the F.normalize eps clamp folded into the ACT bias
    port), then sca = 30/sqrt(ss) = exp(-0.5*ln + ln30).
  ACT activation tables: bass pins each activation function to the first
    act_info.json set containing it, which makes Ln<->Exp alternation
    reload tables ~2x per block (~2.7us each). natural_log_exp_and_others
    holds ALL functions used here (exp/ln/square/identity), so build()
    patches the table registry handed to bacc's load-insertion pass to
    pin everything to that one set: exactly one ACT_TABLE_LOAD per run.
  Block 0 uses finer chunks (DVE) plus a bigger ACT share so pass 1
    trails the very first DMAs; the first big exp starts ~31us in.
  All label/margin correction math is batched AFTER the block loop (one
    FD=8 pass per op) - per-block corrections would stall ACT behind
    3.5us-a-pop GpSimd scalar ops.
The margin term needs only x[r, label_r], gathered once per core with a
1024-element indirect DMA; the label column of the softmax sum is corrected
analytically: S' = S - exp(30t) + exp(30t - 15), t = x_label/||x||.
nll = ln(S') - (30t - 15).  Per-core scalar partial via a [128,1]x[128,1]
matmul against a 1/N vector; the host unshard sums the 8 per-core partials
(a device-side AllReduce of the 4-byte scalar costs ~55us of ncfw floor).
"""

import math

import numpy as np

MARGIN = 0.5
SCALE = 30.0
N_CORES = 8
N_TOTAL = 8192
C = 32000
P = 128

R = N_TOTAL // N_CORES  # rows per core
B = R // P  # row blocks per core

# per-block column plan: list of (engine, offset, width); 'a' = ACT Square,
# 'v' = VectorE stt.  DMA issue order == list order.  The ACT chunk goes
# LAST: it lands at the end of the block's DMA window, exactly when ACT
# (running one block behind) gets to it - so the ss chain never waits on
# a late DVE stt of a late-landing chunk.
_STEADY = [
    ("v", 0, 10200),
    ("v", 10200, 10200),
    ("v", 20400, 10000),
    ("a", 30400, 1600),
]
# block 0: v-chunks first (DVE starts as soon as each lands), ACT sweeps the
# back half while the initial DMA stream finishes
_BLOCK0 = [
    ("v", 0, 8000),
    ("v", 8000, 8000),
    ("v", 16000, 3200),
    ("a", 19200, 3200),
    ("a", 22400, 3200),
    ("a", 25600, 3200),
    ("a", 28800, 3200),
]
SPANS_BY_BLOCK = [_BLOCK0] + [_STEADY] * (B - 1)


def _pin_act_tables(bacc_mod, mybir):
    """Patch the activation-table registry handed to bacc's table-load
    insertion so every function this kernel uses pins to the one set that
    contains them all (natural_log_exp_and_others). Set ids (dict order)
    are preserved; other sets merely stop advertising these functions."""
    AF = mybir.ActivationFunctionType
    orig = bacc_mod.get_activation_tables
    if getattr(orig, "_aam_pinned", False):
        return
    pinned_funcs = {AF.Exp, AF.Ln, AF.Square, AF.Identity}
    keep = "natural_log_exp_and_others"

    def patched(arch):
        t = dict(orig(arch))
        if keep in t:
            for k in t:
                if k != keep:
                    t[k] = set(t[k]) - pinned_funcs
        return t

    patched._aam_pinned = True
    bacc_mod.get_activation_tables = patched


def build(
    n_rows=R,
    n_cols=C,
    n_cores=N_CORES,
    n_total=N_TOTAL,
    v_bufs=6,
    a_bufs=6,
):
    """Build + compile the per-core Bass graph (SPMD, identical on all cores)."""
    import concourse.bacc as bacc
    import concourse.bass as bass
    import concourse.tile as tile
    from concourse import mybir

    f32 = mybir.dt.float32
    bf16 = mybir.dt.bfloat16
    u32 = mybir.dt.uint32
    AF = mybir.ActivationFunctionType
    ALU = mybir.AluOpType
    AX = mybir.AxisListType

    _pin_act_tables(bacc, mybir)

    b_blocks = n_rows // P
    assert n_rows % P == 0
    for spans in SPANS_BY_BLOCK:
        assert sum(w for _, _, w in spans) == n_cols
    n_es = sum(len(s) for s in SPANS_BY_BLOCK)
    v_max = max(w for s in SPANS_BY_BLOCK for e, _, w in s if e == "v")
    a_max = max(w for s in SPANS_BY_BLOCK for e, _, w in s if e == "a")

    nc = bacc.Bacc("TRN2", target_bir_lowering=False, debug=False, num_devices=n_cores)

    logits_ext = nc.dram_tensor("logits", [n_rows, n_cols], bf16, kind="ExternalInput")
    goff_ext = nc.dram_tensor("goff", [P, b_blocks], u32, kind="ExternalInput")
    # per-(partition, block) partials of (lse - t30); host sums and adds 15
    out_ext = nc.dram_tensor("out", [P, b_blocks], f32, kind="ExternalOutput")

    neg_m = -SCALE * MARGIN  # -15
    ln_s = math.log(SCALE)

    with tile.TileContext(nc) as tc:
        with (
            tc.tile_pool(name="chunks", bufs=1) as chunks,
            tc.tile_pool(name="singles", bufs=1) as singles,
            tc.tile_pool(name="smalls", bufs=3) as smalls,
        ):
            # label-logit gather: one indirect DMA for all rows of this core
            # (goff via gpsimd/SWDGE so the sync HWDGE queue leads with the
            # first streaming chunk)
            goff_sb = singles.tile([P, b_blocks], u32)
            nc.gpsimd.dma_start(out=goff_sb[:, :], in_=goff_ext[:, :])
            xl_all = singles.tile([P, b_blocks], bf16)
            logits_flat = logits_ext.ap().rearrange("r (c one) -> (r c) one", one=1)
            nc.gpsimd.indirect_dma_start(
                out=xl_all[:, :],
                out_offset=None,
                in_=logits_flat,
                in_offset=bass.IndirectOffsetOnAxis(ap=goff_sb[:, :], axis=0),
            )

            zero_t = singles.tile([P, 1], f32)
            nc.vector.memset(zero_t, 0.0)
            m15_t = singles.tile([P, 1], f32)
            nc.vector.memset(m15_t, neg_m)
            ln30_t = singles.tile([P, 1], f32)
            nc.vector.memset(ln30_t, ln_s)
            eps2_t = singles.tile([P, 1], f32)
            nc.vector.memset(eps2_t, 1e-24)

            # warm-up: trigger the single ACT table load during the DMA ramp
            warm = singles.tile([P, 1], f32)
            nc.scalar.activation(out=warm[:, :], in_=zero_t[:, :], func=AF.Exp, bias=zero_t[:, :])

            # persistent per-block state for the batched tail
            sca_all = singles.tile([P, b_blocks], f32)
            es_all = singles.tile([P, n_es], f32)
            s_sum = singles.tile([P, b_blocks], f32)
            # stt needs a full-size dummy out (never read)
            dump_v = singles.tile([P, v_max], bf16)
            dump_a = singles.tile([P, a_max], bf16)

            es_bases = []
            es_base = 0
            for spans in SPANS_BY_BLOCK:
                es_bases.append(es_base)
                es_base += len(spans)

            def s_sum_reduce(bb):
                nc.vector.reduce_sum(
                    out=s_sum[:, bb : bb + 1],
                    in_=es_all[:, es_bases[bb] : es_bases[bb] + len(SPANS_BY_BLOCK[bb])],
                    axis=AX.X,
                )

            for b, spans in enumerate(SPANS_BY_BLOCK):
                es_base = es_bases[b]
                ncol = len(spans)
                rs = b * P
                ss_cols = smalls.tile([P, ncol], f32, tag="ss_cols", name=f"ssc_{b}")
                chs = []
                for eng, off, w in spans:
                    # rings by width class so SBUF isn't wasted on padding
                    if w > 3200:
                        tag, bufs, wmax = "vch", v_bufs, 10200
                    else:
                        tag, bufs, wmax = "ach", a_bufs, 3200
                    ch = chunks.tile([P, wmax], bf16, tag=tag, bufs=bufs, name=f"c{b}_{off}")
                    nc.sync.dma_start(
                        out=ch[:, :w], in_=logits_ext[rs : rs + P, off : off + w]
                    )
                    chs.append((eng, ch, w))
                # pass 1: ss_cols[:, i] = sum(chunk^2)
                for i, (eng, ch, w) in enumerate(chs):
                    if eng == "a":
                        nc.scalar.activation(
                            out=dump_a[:, :w],
                            in_=ch[:, :w],
                            func=AF.Square,
                            bias=zero_t[:, :],
                            accum_out=ss_cols[:, i : i + 1],
                        )
                    else:
                        nc.vector.scalar_tensor_tensor(
                            out=dump_v[:, :w],
                            in0=ch[:, :w],
                            scalar=1.0,
                            in1=ch[:, :w],
                            op0=ALU.mult,
                            op1=ALU.mult,
                            accum_out=ss_cols[:, i : i + 1],
                        )

                # whole scale chain on ACT (no cross-engine ping-pong):
                # ss = sum(ss_cols); u = ln(ss + eps^2); sca = exp(-u/2 + ln30)
                ss_dump = smalls.tile([P, ncol], f32, tag="ss_dump", name=f"ssd_{b}")
                ss = smalls.tile([P, 1], f32, tag="ss")
                nc.scalar.activation(
                    out=ss_dump[:, :],
                    in_=ss_cols[:, :],
                    func=AF.Identity,
                    bias=zero_t[:, :],
                    accum_out=ss[:, :],
                )
                u = smalls.tile([P, 1], f32, tag="u")
                nc.scalar.activation(out=u[:, :], in_=ss[:, :], func=AF.Ln, bias=eps2_t[:, :])
                nc.scalar.activation(
                    out=sca_all[:, b : b + 1],
                    in_=u[:, :],
                    func=AF.Exp,
                    bias=ln30_t[:, :],
                    scale=-0.5,
                )

                # pass 2: es_all[:, es_base+i] = sum(exp(sca * x)), in place
                for i, (eng, ch, w) in enumerate(chs):
                    col = es_base + i
                    nc.scalar.activation(
                        out=ch[:, :w],
                        in_=ch[:, :w],
                        func=AF.Exp,
                        bias=zero_t[:, :],
                        scale=sca_all[:, b : b + 1],
                        accum_out=es_all[:, col : col + 1],
                    )
                # overlap the tail's per-block es reduction: by now block b-3's
                # exps finished long ago, so this never stalls the DVE queue
                if b >= 3:
                    s_sum_reduce(b - 3)

            # ---- batched tail: margin/label correction for all blocks ----
            for bb in range(max(0, b_blocks - 3), b_blocks):
                s_sum_reduce(bb)
            # t30 = 30 * x_label / ||x||
            t30 = singles.tile([P, b_blocks], f32)
            nc.vector.tensor_tensor(
                out=t30[:, :], in0=xl_all[:, :], in1=sca_all[:, :], op=ALU.mult
            )
            e1 = singles.tile([P, b_blocks], f32)
            nc.scalar.activation(out=e1[:, :], in_=t30[:, :], func=AF.Exp, bias=zero_t[:, :])
            e2 = singles.tile([P, b_blocks], f32)
            nc.scalar.activation(out=e2[:, :], in_=t30[:, :], func=AF.Exp, bias=m15_t[:, :])
            # sc = s_sum - e1 + e2  (replace label term with margined one)
            sc1 = singles.tile([P, b_blocks], f32)
            nc.vector.scalar_tensor_tensor(
                out=sc1[:, :], in0=e1[:, :], scalar=-1.0, in1=s_sum[:, :],
                op0=ALU.mult, op1=ALU.add,
            )
            sc2 = singles.tile([P, b_blocks], f32)
            nc.vector.tensor_tensor(out=sc2[:, :], in0=sc1[:, :], in1=e2[:, :], op=ALU.add)
            lse = singles.tile([P, b_blocks], f32)
            nc.scalar.activation(out=lse[:, :], in_=sc2[:, :], func=AF.Ln, bias=zero_t[:, :])
            # nll0 = lse - t30; the host adds the constant +15 and divides by N
            nll0 = singles.tile([P, b_blocks], f32)
            nc.vector.scalar_tensor_tensor(
                out=nll0[:, :], in0=t30[:, :], scalar=-1.0, in1=lse[:, :],
                op0=ALU.mult, op1=ALU.add,
            )
            nc.sync.dma_start(out=out_ext[:, :], in_=nll0[:, :])

    nc.compile()
    return nc


_NC_CACHE = None


def _get_nc():
    global _NC_CACHE
    if _NC_CACHE is None:
        _NC_CACHE = build()
    return _NC_CACHE


def make_in_maps(logits, labels):
    import ml_dtypes

    logits = np.asarray(logits, dtype=np.float32)
    labels = np.asarray(labels).astype(np.int64)
    assert logits.shape == (N_TOTAL, C), logits.shape
    logits_bf16 = logits.astype(ml_dtypes.bfloat16)
    in_maps = []
    for i in range(N_CORES):
        shard = np.ascontiguousarray(logits_bf16[i * R : (i + 1) * R])
        lab = labels[i * R : (i + 1) * R]
        flat = np.arange(R, dtype=np.int64) * C + lab  # local flat element index
        goff = np.ascontiguousarray(flat.reshape(B, P).T).astype(np.uint32)
        in_maps.append({"logits": shard, "goff": goff})
    return in_maps


def unshard(results):
    # each core emits [128, B] partials of (lse - t30); loss = 15 + sum/N
    acc = 0.0
    for r in results:
        acc += float(np.asarray(r["out"], dtype=np.float32).sum(dtype=np.float64))
    return np.array(SCALE * MARGIN + acc / N_TOTAL, dtype=np.float32)


def kernel(**inputs):
    from concourse.bass_utils import run_bass_kernel_spmd

    nc = _get_nc()
    in_maps = make_in_maps(inputs["logits"], inputs["labels"])
    res = run_bass_kernel_spmd(nc, in_maps, core_ids=list(range(N_CORES)))
    return unshard(res.results)
